# revision 1
# baseline (speedup 1.0000x reference)
"""ContentOnlyRouter MoE kernel for 8x TRN2 NeuronCores.

Strategy (expert-parallel, two SPMD launches):
  Launch A (data-parallel over tokens): each core scores its 2048-token shard
    against sign(tile_sigs) and computes per-token argmax expert ids.
    Scoring uses a bf16 hi/lo split of x (products with +-1 are exact in bf16;
    fp32 PSUM accumulation) so the argmax matches fp32 scoring exactly.
  Host glue: stable counting-sort of the 16384 expert ids (64KB of metadata)
    to build per-expert gather lists.
  Launch B (expert-parallel): core t owns expert t. dma_gather(transpose=True)
    pulls its ~2048 assigned token rows from a replicated bf16 copy of x and
    transposes them on the fly into [d, tok] matmul layout. 8 accumulating
    bf16 matmuls per 128-token block compute x @ W[t], bias added on DVE,
    fp32 rows stored compactly. Host scatters rows back to token order.

Shapes are hardcoded for B=4, S=4096, D=1024, T=8 per the problem spec.
"""

import os

os.environ.setdefault("JAX_PLATFORMS", "")

import numpy as np
import ml_dtypes

import concourse.bass as bass
import concourse.bacc as bacc
import concourse.mybir as mybir
import concourse.tile as tile
from concourse.masks import make_identity

B, S, D, T = 4, 4096, 1024, 8
NTOK = B * S            # 16384 tokens
NG = 4                  # score groups of 512 tokens per shard
NCORES = 8
SHARD = NTOK // NCORES  # 2048 tokens scored per core
CAP = 2304              # per-expert token capacity (18 blocks of 128)
GCHUNK = 384            # tokens per dma_gather call (3 blocks of 128)
NCHUNK = CAP // GCHUNK  # 6
TRASH = NTOK            # row index used for padding slots
DC = D // 128           # 8 contraction chunks

F32 = mybir.dt.float32
BF16 = mybir.dt.bfloat16
I16 = mybir.dt.int16

_perf = []  # exec_time_ns per launch when tracing


def build_launch_a(iters=1):
    """Scores + argmax for one 2048-token shard."""
    nc = bacc.Bacc(None)
    xht = nc.dram_tensor("xht", [128, DC, SHARD], BF16, kind="ExternalInput")
    xlt = nc.dram_tensor("xlt", [128, DC, SHARD], BF16, kind="ExternalInput")
    sgn = nc.dram_tensor("sgn", [128, DC, T], BF16, kind="ExternalInput")
    idx = nc.dram_tensor("idx", [SHARD], F32, kind="ExternalOutput")

    with tile.TileContext(nc) as tc:
        with (
            tc.tile_pool(name="const", bufs=1) as const,
            tc.tile_pool(name="xa", bufs=4) as xa,
            tc.tile_pool(name="ps", bufs=2, space="PSUM") as ps,
            tc.tile_pool(name="pst", bufs=4, space="PSUM") as pst,
            tc.tile_pool(name="sb", bufs=2) as sb,
        ):
            sgn_sb = const.tile([128, DC, T], BF16)
            nc.sync.dma_start(out=sgn_sb, in_=sgn[:, :, :])
            ident = const.tile([128, 128], F32)
            make_identity(nc, ident)
            # rev-iota: value 7-t at expert slot t (first-occurrence argmax)
            revio = const.tile([128, NG * 4, T], F32)
            for t in range(T):
                nc.vector.memset(revio[:, :, t : t + 1], float(T - 1 - t))
            sc_all = const.tile([128, NG * 4, T], F32)

            import contextlib
            loop = tc.For_i(0, iters, 1) if iters > 1 else contextlib.nullcontext()
            with loop:
                self_body_a(nc, tc, xa, ps, pst, sb, sgn_sb, ident, revio, sc_all, xht, xlt, idx)
    nc.compile()
    return nc


def self_body_a(nc, tc, xa, ps, pst, sb, sgn_sb, ident, revio, sc_all, xht, xlt, idx):
    if True:
            for g in range(NG):
                xh_g = xa.tile([128, DC, 512], BF16, tag="xh")
                xl_g = xa.tile([128, DC, 512], BF16, tag="xl")
                nc.sync.dma_start(out=xh_g, in_=xht[:, :, 512 * g : 512 * (g + 1)])
                nc.sync.dma_start(out=xl_g, in_=xlt[:, :, 512 * g : 512 * (g + 1)])
                psum_s = ps.tile([T, 512], F32)
                for c in range(DC):
                    nc.tensor.matmul(
                        out=psum_s,
                        lhsT=sgn_sb[:, c, :],
                        rhs=xh_g[:, c, :],
                        start=(c == 0),
                        stop=False,
                    )
                for c in range(DC):
                    nc.tensor.matmul(
                        out=psum_s,
                        lhsT=sgn_sb[:, c, :],
                        rhs=xl_g[:, c, :],
                        start=False,
                        stop=(c == DC - 1),
                    )
                s_sb = sb.tile([T, 512], F32)
                nc.vector.tensor_copy(out=s_sb, in_=psum_s)
                for j in range(4):
                    p_t = pst.tile([128, T], F32)
                    nc.tensor.transpose(
                        out=p_t,
                        in_=s_sb[:, 128 * j : 128 * (j + 1)],
                        identity=ident[0:T, 0:T],
                    )
                    nc.vector.tensor_copy(out=sc_all[:, 4 * g + j, :], in_=p_t)

            # argmax over the last axis (8 experts) per token
            smax = sb.tile([128, NG * 4, 1], F32, tag="smax")
            nc.vector.reduce_max(out=smax, in_=sc_all, axis=mybir.AxisListType.X)
            m = sb.tile([128, NG * 4, T], F32, tag="m")
            nc.vector.tensor_tensor(
                out=m,
                in0=sc_all,
                in1=smax.to_broadcast([128, NG * 4, T]),
                op=mybir.AluOpType.is_ge,
            )
            nc.vector.tensor_tensor(out=m, in0=m, in1=revio, op=mybir.AluOpType.mult)
            mm = sb.tile([128, NG * 4, 1], F32, tag="mm")
            nc.vector.reduce_max(out=mm, in_=m, axis=mybir.AxisListType.X)
            idxv = sb.tile([128, NG * 4], F32, tag="idxv")
            nc.vector.tensor_scalar(
                out=idxv,
                in0=mm[:, :, 0],
                scalar1=-1.0,
                scalar2=float(T - 1),
                op0=mybir.AluOpType.mult,
                op1=mybir.AluOpType.add,
            )
            # token n = 128*q + p  ->  idx[n]
            nc.sync.dma_start(
                out=idx.rearrange("(q p) -> p q", p=128), in_=idxv
            )


def build_launch_b(iters=1):
    """Gather + expert matmul for one expert's tokens."""
    nc = bacc.Bacc(None)
    xfull = nc.dram_tensor("xfull", [NTOK + 1, D], BF16, kind="ExternalInput")
    wt = nc.dram_tensor("wt", [128, DC, D], BF16, kind="ExternalInput")
    bt = nc.dram_tensor("bt", [D], F32, kind="ExternalInput")
    gl = nc.dram_tensor("gl", [128, CAP // 16], I16, kind="ExternalInput")
    orows = nc.dram_tensor("orows", [CAP, D], F32, kind="ExternalOutput")

    with tile.TileContext(nc) as tc:
        with (
            tc.tile_pool(name="const", bufs=1) as const,
            tc.tile_pool(name="gx", bufs=3) as gxp,
            tc.tile_pool(name="ps", bufs=4, space="PSUM") as ps,
            tc.tile_pool(name="osb", bufs=3) as osb,
        ):
            w_sb = const.tile([128, DC, D], BF16)
            nc.sync.dma_start(out=w_sb, in_=wt[:, :, :])
            b_sb = const.tile([128, D], F32)
            bt_ap = bt[:]
            nc.gpsimd.dma_start(
                out=b_sb,
                in_=bass.AP(
                    tensor=bt_ap.tensor, offset=bt_ap.offset,
                    ap=[[0, 128]] + list(bt_ap.ap),
                ),
            )
            gl_sb = const.tile([128, CAP // 16], I16)
            nc.sync.dma_start(out=gl_sb, in_=gl[:, :])

            import contextlib
            loop = tc.For_i(0, iters, 1) if iters > 1 else contextlib.nullcontext()
            with loop:
                self_body_b(nc, tc, gxp, ps, osb, w_sb, b_sb, gl_sb, xfull, orows)
    nc.compile()
    return nc


def self_body_b(nc, tc, gxp, ps, osb, w_sb, b_sb, gl_sb, xfull, orows):
    if True:
            for ch in range(NCHUNK):
                gx = gxp.tile([128, DC, GCHUNK], BF16)
                nc.gpsimd.dma_gather(
                    out_ap=gx,
                    in_ap=xfull[:, :],
                    idxs_ap=gl_sb[:, (GCHUNK // 16) * ch : (GCHUNK // 16) * (ch + 1)],
                    num_idxs=GCHUNK,
                    num_idxs_reg=GCHUNK,
                    elem_size=D,
                    transpose=True,
                )
                for blk in range(GCHUNK // 128):
                    tok = slice(128 * blk, 128 * (blk + 1))
                    ps0 = ps.tile([128, 512], F32, tag="ps0")
                    ps1 = ps.tile([128, 512], F32, tag="ps1")
                    for c in range(DC):
                        nc.tensor.matmul(
                            out=ps0,
                            lhsT=gx[:, c, tok],
                            rhs=w_sb[:, c, 0:512],
                            start=(c == 0),
                            stop=(c == DC - 1),
                        )
                        nc.tensor.matmul(
                            out=ps1,
                            lhsT=gx[:, c, tok],
                            rhs=w_sb[:, c, 512:1024],
                            start=(c == 0),
                            stop=(c == DC - 1),
                        )
                    o_t = osb.tile([128, D], F32)
                    nc.vector.tensor_add(out=o_t[:, 0:512], in0=ps0, in1=b_sb[:, 0:512])
                    nc.vector.tensor_add(out=o_t[:, 512:1024], in0=ps1, in1=b_sb[:, 512:1024])
                    row0 = GCHUNK * ch + 128 * blk
                    nc.sync.dma_start(out=orows[row0 : row0 + 128, :], in_=o_t)


_nc_a = None
_nc_b = None


def _get_programs():
    global _nc_a, _nc_b
    if _nc_a is None:
        _nc_a = build_launch_a()
        _nc_b = build_launch_b()
    return _nc_a, _nc_b


def _run_spmd(nc, in_maps, label):
    if os.environ.get("BASS_SIM"):
        from concourse.bass_interp import CoreSim

        results = []
        for im in in_maps:
            sim = CoreSim(nc)
            for k, v in im.items():
                sim.tensor(k)[:] = v
            sim.simulate()
            out = {}
            for alloc in nc.m.functions[0].allocations:
                if getattr(alloc, "kind", None) == "ExternalOutput":
                    name = alloc.memorylocations[0].name
                    out[name] = np.array(sim.mem_tensor(name))
            results.append(out)

        class R:
            pass

        r = R()
        r.results = results
        r.exec_time_ns = None
        return r
    from concourse.bass_utils import run_bass_kernel_spmd

    trace = bool(os.environ.get("BASS_TRACE"))
    kw = {}
    if trace:
        tdir = os.path.abspath(f"trace_{label}")
        os.makedirs(tdir, exist_ok=True)
        kw = dict(trace=True, tmpdir=tdir, trace_cores=[0])
    res = run_bass_kernel_spmd(nc, in_maps, core_ids=list(range(NCORES)), **kw)
    if trace:
        _perf.append((label, res.exec_time_ns, res.mean_exec_time_ns))
    return res


def kernel(x, tile_sigs, W, b):
    x = np.asarray(x, np.float32)
    tile_sigs = np.asarray(tile_sigs, np.float32)
    W = np.asarray(W, np.float32)
    b = np.asarray(b, np.float32)
    _perf.clear()

    nc_a, nc_b = _get_programs()

    xf = x.reshape(NTOK, D)
    x_hi = xf.astype(ml_dtypes.bfloat16)
    x_lo = (xf - x_hi.astype(np.float32)).astype(ml_dtypes.bfloat16)
    sgn = np.sign(tile_sigs).astype(ml_dtypes.bfloat16)  # [T, D]
    # sgn_in[p, c, t] = sgn[t, 128c + p]
    sgn_in = np.ascontiguousarray(sgn.T.reshape(DC, 128, T).transpose(1, 0, 2))

    in_maps_a = []
    for c in range(NCORES):
        sh = slice(c * SHARD, (c + 1) * SHARD)
        # xht[p, ch, n] = x_hi[n, 128*ch + p]
        xht = np.ascontiguousarray(x_hi[sh].T.reshape(DC, 128, SHARD).transpose(1, 0, 2))
        xlt = np.ascontiguousarray(x_lo[sh].T.reshape(DC, 128, SHARD).transpose(1, 0, 2))
        in_maps_a.append({"xht": xht, "xlt": xlt, "sgn": sgn_in})

    res_a = _run_spmd(nc_a, in_maps_a, "a")
    idx_all = np.concatenate(
        [np.rint(res_a.results[c]["idx"]).astype(np.int64).ravel() for c in range(NCORES)]
    )

    # host routing: stable counting sort -> per-expert gather lists
    order = np.argsort(idx_all, kind="stable")
    counts = np.bincount(idx_all, minlength=T)
    assert counts.max() <= CAP, f"expert overflow: {counts}"
    bounds = np.concatenate([[0], np.cumsum(counts)])

    x_hi_full = np.vstack([x_hi, np.zeros((1, D), ml_dtypes.bfloat16)])
    gids = []
    in_maps_b = []
    for t in range(NCORES):
        ids = order[bounds[t] : bounds[t + 1]]
        glf = np.full(CAP, TRASH, np.int64)
        glf[: len(ids)] = ids
        gids.append(glf)
        wrapped = np.ascontiguousarray(
            glf.reshape(CAP // 16, 16).T.astype(np.int16)
        )  # [16, CAP//16]
        gl_in = np.tile(wrapped, (8, 1))  # replicate for 8 gpsimd cores
        # wt[p, c, e] = W[t][128c + p, e]
        wt = np.ascontiguousarray(
            W[t].astype(ml_dtypes.bfloat16).reshape(DC, 128, D).transpose(1, 0, 2)
        )
        in_maps_b.append({"xfull": x_hi_full, "wt": wt, "bt": b[t], "gl": gl_in})

    res_b = _run_spmd(nc_b, in_maps_b, "b")

    out_full = np.zeros((NTOK + 1, D), np.float32)
    for t in range(NCORES):
        out_full[gids[t]] = res_b.results[t]["orows"]
    return out_full[:NTOK].reshape(B, S, D)



# revision 7
# speedup vs baseline: 1.3417x; 1.3417x over previous
"""ContentOnlyRouter MoE kernel for 8x TRN2 NeuronCores.

Strategy (two SPMD launches + host routing glue):
  Launch A (data-parallel scoring): each core scores its 2048-token shard
    against sign(tile_sigs) using fp8-e4m3 x with DoubleRow matmuls (2x PE
    rate), transposes the [8, tok] psum scores into token-partition layout
    and DMAs the raw fp32 scores out. Scores are exact sums of +-fp8(x), so
    the host knows their error bound vs the true fp32 scores.
  Host glue: argmax over device scores; tokens whose top-2 gap < GAP_THR
    are re-scored exactly in float64 (the reference's own fp32-einsum argmax
    matches float64 argmax for every token because the minimum true top-2
    gap is ~1e-3 while fp32 scoring noise is ~6e-5). Then tokens are packed
    into 16 weight slots (8 cores x [A=1152, B=1024] rows) such that each
    slot holds tokens of a single expert - every core runs exactly 17
    blocks of 128 tokens regardless of expert imbalance.
  Launch B (slot-parallel expert matmul): per core, gather its two slots'
    token rows from a replicated fp16 copy of x (dma_gather transpose=True
    into [d, tok] matmul layout), run 16 accumulating fp16 matmuls per
    128-token block against the slot's weights, add bias on DVE, store
    bf16 rows. Host scatters rows back to token order.

Shapes hardcoded for B=4, S=4096, D=1024, T=8 per the problem spec.
"""

import os

os.environ.setdefault("JAX_PLATFORMS", "")

import numpy as np
import ml_dtypes

import concourse.bass as bass
import concourse.bacc as bacc
import concourse.mybir as mybir
import concourse.tile as tile
from concourse.masks import make_identity

B, S, D, T = 4, 4096, 1024, 8
NTOK = B * S            # 16384 tokens
NCORES = 8
SHARD = NTOK // NCORES  # 2048 tokens scored per core
NG = 4                  # score groups of 512 tokens per shard
DC = D // 128           # 8 contraction chunks of 128

CAPA = 1152             # slot A: 9 blocks of 128
CAPB = 1024             # slot B: 8 blocks of 128
CAP = CAPA + CAPB       # 17 blocks per core
TRASH = NTOK            # row index used for padding slots (zero row)
GAP_THR = 6.0           # fp8 score top-2 gap below which host re-scores

F32 = mybir.dt.float32
FP16 = mybir.dt.float16
BF16 = mybir.dt.bfloat16
FP8 = mybir.dt.float8e4
I16 = mybir.dt.int16

NPFP8 = ml_dtypes.float8_e4m3

_perf = []  # (label, exec_time_ns, mean) per launch when tracing


def build_launch_a(iters=1):
    """fp8 DoubleRow scoring for one 2048-token shard; outputs raw scores."""
    nc = bacc.Bacc(None)
    x8 = nc.dram_tensor("x8", [128, DC, SHARD], FP8, kind="ExternalInput")
    sg8 = nc.dram_tensor("sg8", [128, DC, 16], FP8, kind="ExternalInput")
    # sct[p, q, t] = score(token 128*q + p, expert t)
    sct = nc.dram_tensor("sct", [128, SHARD // 128, T], F32, kind="ExternalOutput")

    with tile.TileContext(nc) as tc:
        with (
            tc.tile_pool(name="const", bufs=1) as const,
            tc.tile_pool(name="xa", bufs=2) as xa,
            tc.tile_pool(name="ps", bufs=2, space="PSUM") as ps,
            tc.tile_pool(name="pst", bufs=4, space="PSUM") as pst,
            tc.tile_pool(name="sb", bufs=2) as sb,
        ):
            sg_sb = const.tile([128, DC, 16], FP8)
            nc.sync.dma_start(out=sg_sb, in_=sg8[:, :, :])
            ident = const.tile([128, 128], F32)
            make_identity(nc, ident)
            sc_all = const.tile([128, NG * 4, T], F32)

            import contextlib
            loop = tc.For_i(0, iters, 1) if iters > 1 else contextlib.nullcontext()
            with loop:
                _body_a(nc, tc, xa, ps, pst, sb, sg_sb, ident, sc_all, x8, sct)
    nc.compile()
    return nc


def _body_a(nc, tc, xa, ps, pst, sb, sg_sb, ident, sc_all, x8, sct):
    for g in range(NG):
        xg = xa.tile([128, DC, 512], FP8, tag="xg")
        nc.sync.dma_start(out=xg, in_=x8[:, :, 512 * g : 512 * (g + 1)])
        psum_s = ps.tile([16, 512], F32, tag="psum_s")
        for c in range(DC // 2):
            nc.tensor.matmul(
                out=psum_s,
                lhsT=sg_sb[:, 2 * c : 2 * c + 2, :],
                rhs=xg[:, 2 * c : 2 * c + 2, :],
                perf_mode=mybir.MatmulPerfMode.DoubleRow,
                start=(c == 0),
                stop=(c == DC // 2 - 1),
            )
        s_sb = sb.tile([T, 512], F32, tag="s_sb")
        nc.vector.tensor_copy(out=s_sb, in_=psum_s[0:T, :])
        for j in range(4):
            p_t = pst.tile([128, T], F32, tag="p_t")
            nc.tensor.transpose(
                out=p_t,
                in_=s_sb[:, 128 * j : 128 * (j + 1)],
                identity=ident[0:T, 0:T],
            )
            nc.vector.tensor_copy(out=sc_all[:, 4 * g + j, :], in_=p_t)
    nc.sync.dma_start(out=sct[:, :, :], in_=sc_all)


def build_launch_b(iters=1):
    """Gather + two-slot expert matmul (9 + 8 blocks of 128 tokens)."""
    nc = bacc.Bacc(None)
    xfull = nc.dram_tensor("xfull", [NTOK + 1, D], FP16, kind="ExternalInput")
    wa = nc.dram_tensor("wa", [128, DC, D], FP16, kind="ExternalInput")
    wb = nc.dram_tensor("wb", [128, DC, D], FP16, kind="ExternalInput")
    ba = nc.dram_tensor("ba", [1, D], F32, kind="ExternalInput")
    bb = nc.dram_tensor("bb", [1, D], F32, kind="ExternalInput")
    gla = nc.dram_tensor("gla", [128, CAPA // 16], I16, kind="ExternalInput")
    glb = nc.dram_tensor("glb", [128, CAPB // 16], I16, kind="ExternalInput")
    orows = nc.dram_tensor("orows", [CAP, D], BF16, kind="ExternalOutput")

    with tile.TileContext(nc) as tc:
        with (
            tc.tile_pool(name="const", bufs=1) as const,
            tc.tile_pool(name="gxa", bufs=2) as gxa,
            tc.tile_pool(name="gxb", bufs=2) as gxb,
            tc.tile_pool(name="ps", bufs=4, space="PSUM") as ps,
            tc.tile_pool(name="osb", bufs=3) as osb,
        ):
            gla_sb = const.tile([128, CAPA // 16], I16)
            nc.sync.dma_start(out=gla_sb, in_=gla[:, :])
            glb_sb = const.tile([128, CAPB // 16], I16)
            nc.sync.dma_start(out=glb_sb, in_=glb[:, :])

            # bias rows + fp16 ones column for PE partition-broadcast
            ones = const.tile([1, 128], FP16)
            nc.vector.memset(ones, 1.0)
            brow16 = {}
            for slot, src in (("a", ba), ("b", bb)):
                br = const.tile([1, D], F32, tag=f"brow{slot}", name=f"brow{slot}")
                nc.sync.dma_start(out=br, in_=src[:, :])
                br16 = const.tile([1, D], FP16, tag=f"brow16{slot}", name=f"brow16{slot}")
                nc.vector.tensor_copy(out=br16, in_=br)
                brow16[slot] = br16

            w_sb = {}
            w_sb["a"] = const.tile([128, DC, D], FP16, tag="wa_sb", name="wa_sb")
            w_sb["b"] = const.tile([128, DC, D], FP16, tag="wb_sb", name="wb_sb")
            for c in range(DC):
                nc.sync.dma_start(out=w_sb["a"][:, c, :], in_=wa[:, c, :])
            for c in range(DC):
                nc.sync.dma_start(out=w_sb["b"][:, c, :], in_=wb[:, c, :])

            # broadcast biases across partitions: ones[1,128].T @ brow16[1,:]
            b_bc = {}
            for slot in ("a", "b"):
                pb0 = ps.tile([128, 512], F32, tag="ps0", name=f"pb0{slot}")
                pb1 = ps.tile([128, 512], F32, tag="ps1", name=f"pb1{slot}")
                nc.tensor.matmul(out=pb0, lhsT=ones,
                                 rhs=brow16[slot][:, 0:512])
                nc.tensor.matmul(out=pb1, lhsT=ones,
                                 rhs=brow16[slot][:, 512:1024])
                bt = const.tile([128, D], F32, tag=f"bbc{slot}", name=f"bbc{slot}")
                nc.vector.tensor_copy(out=bt[:, 0:512], in_=pb0)
                nc.vector.tensor_copy(out=bt[:, 512:1024], in_=pb1)
                b_bc[slot] = bt

            import contextlib
            loop = tc.For_i(0, iters, 1) if iters > 1 else contextlib.nullcontext()
            with loop:
                _body_b(nc, tc, gxa, gxb, ps, osb, w_sb, b_bc,
                        gla_sb, glb_sb, xfull, orows)
    nc.compile()
    return nc


def _body_b(nc, tc, gxa, gxb, ps, osb, w_sb, b_bc, gla_sb, glb_sb, xfull, orows):
    # slot A: 3 gather chunks x 384 tokens (3 blocks); slot B: 2 x 512 (4 blocks)
    plan = [("a", gxa, gla_sb, 384, 3, 0), ("b", gxb, glb_sb, 512, 2, CAPA)]
    for slot, gxp, gl_sb, gchunk, nchunk, row_base in plan:
        for ch in range(nchunk):
            gx = gxp.tile([128, DC, gchunk], FP16, tag=f"gx{slot}", name=f"gx{slot}")
            nc.gpsimd.dma_gather(
                out_ap=gx,
                in_ap=xfull[:, :],
                idxs_ap=gl_sb[:, (gchunk // 16) * ch : (gchunk // 16) * (ch + 1)],
                num_idxs=gchunk,
                num_idxs_reg=gchunk,
                elem_size=D,
                transpose=True,
            )
            for blk in range(gchunk // 128):
                tok = slice(128 * blk, 128 * (blk + 1))
                ps0 = ps.tile([128, 512], F32, tag="ps0")
                ps1 = ps.tile([128, 512], F32, tag="ps1")
                for c in range(DC):
                    nc.tensor.matmul(
                        out=ps0,
                        lhsT=gx[:, c, tok],
                        rhs=w_sb[slot][:, c, 0:512],
                        start=(c == 0),
                        stop=(c == DC - 1),
                    )
                    nc.tensor.matmul(
                        out=ps1,
                        lhsT=gx[:, c, tok],
                        rhs=w_sb[slot][:, c, 512:1024],
                        start=(c == 0),
                        stop=(c == DC - 1),
                    )
                o_t = osb.tile([128, D], BF16, tag="o_t")
                nc.vector.tensor_add(out=o_t[:, 0:512], in0=ps0,
                                     in1=b_bc[slot][:, 0:512])
                nc.vector.tensor_add(out=o_t[:, 512:1024], in0=ps1,
                                     in1=b_bc[slot][:, 512:1024])
                row0 = row_base + gchunk * ch + 128 * blk
                nc.scalar.dma_start(out=orows[row0 : row0 + 128, :], in_=o_t)


_nc_a = None
_nc_b = None


def _get_programs():
    global _nc_a, _nc_b
    if _nc_a is None:
        _nc_a = build_launch_a()
        _nc_b = build_launch_b()
    return _nc_a, _nc_b


def _run_spmd(nc, in_maps, label):
    if os.environ.get("BASS_SIM"):
        from concourse.bass_interp import CoreSim

        results = []
        for im in in_maps:
            sim = CoreSim(nc)
            for k, v in im.items():
                sim.tensor(k)[:] = v
            sim.simulate()
            out = {}
            for alloc in nc.m.functions[0].allocations:
                if getattr(alloc, "kind", None) == "ExternalOutput":
                    name = alloc.memorylocations[0].name
                    out[name] = np.array(sim.mem_tensor(name))
            results.append(out)

        class R:
            pass

        r = R()
        r.results = results
        r.exec_time_ns = None
        return r
    from concourse.bass_utils import run_bass_kernel_spmd

    trace = bool(os.environ.get("BASS_TRACE"))
    kw = {}
    if trace:
        tdir = os.path.abspath(f"trace_{label}")
        os.makedirs(tdir, exist_ok=True)
        kw = dict(trace=True, tmpdir=tdir, trace_cores=[0])
    res = run_bass_kernel_spmd(nc, in_maps, core_ids=list(range(NCORES)), **kw)
    if trace:
        _perf.append((label, res.exec_time_ns, res.mean_exec_time_ns))
    return res


def _pack_slots(counts):
    """Assign experts to 16 single-expert slots (8 cores x [A=1152, B=1024]).

    Returns (a_own, b_own): expert id owning each core's A / B slot, plus the
    ordered list of (slot refs) per expert used to split its token list.
    """
    big = [t for t in range(T) if counts[t] > CAPA + CAPB]
    assert all(counts[t] <= 2 * CAPA for t in big), f"expert too large: {counts}"
    # smallest len(big) experts (that are not big) run on two B slots
    order_small = sorted((t for t in range(T) if t not in big),
                         key=lambda t: counts[t])
    bb = order_small[: len(big)]
    assert all(counts[t] <= 2 * CAPB for t in bb), f"B+B overflow: {counts}"
    ab = [t for t in range(T) if t not in big and t not in bb]

    a_own, b_own = [], []
    for t in big:
        a_own += [t, t]
    for t in bb:
        b_own += [t, t]
    for t in ab:
        a_own.append(t)
        b_own.append(t)
    assert len(a_own) == NCORES and len(b_own) == NCORES
    # per-expert ordered slot list: ('a'|'b', core, capacity)
    slots_of = {t: [] for t in range(T)}
    for core, t in enumerate(a_own):
        slots_of[t].append(("a", core, CAPA))
    for core, t in enumerate(b_own):
        slots_of[t].append(("b", core, CAPB))
    for t in range(T):
        assert sum(c for _, _, c in slots_of[t]) >= counts[t], f"pack fail {counts}"
    return a_own, b_own, slots_of


def _wrap_gl(ids, cap):
    glf = np.full(cap, TRASH, np.int64)
    glf[: len(ids)] = ids
    wrapped = np.ascontiguousarray(glf.reshape(cap // 16, 16).T.astype(np.int16))
    return np.tile(wrapped, (8, 1)), glf  # [128, cap//16], padded id list


def kernel(x, tile_sigs, W, b):
    x = np.asarray(x, np.float32)
    tile_sigs = np.asarray(tile_sigs, np.float32)
    W = np.asarray(W, np.float32)
    b = np.asarray(b, np.float32)
    _perf.clear()

    nc_a, nc_b = _get_programs()

    xf = x.reshape(NTOK, D)
    x8 = xf.astype(NPFP8)
    sgn = np.sign(tile_sigs)

    # x8t[p, c, n] = x8[tok0 + n, 128c + p]
    sg8_in = np.zeros((128, DC, 16), NPFP8)
    sg8_in[:, :, :T] = sgn.astype(NPFP8).T.reshape(DC, 128, T).transpose(1, 0, 2)
    in_maps_a = []
    for cidx in range(NCORES):
        sh = slice(cidx * SHARD, (cidx + 1) * SHARD)
        x8t = np.ascontiguousarray(x8[sh].T.reshape(DC, 128, SHARD).transpose(1, 0, 2))
        in_maps_a.append({"x8": x8t, "sg8": sg8_in})

    res_a = _run_spmd(nc_a, in_maps_a, "a")

    # assemble scores: sct[p, q, t] -> scores[2048*core + 128q + p, t]
    scores = np.concatenate(
        [
            np.asarray(res_a.results[cidx]["sct"], np.float32)
            .transpose(1, 0, 2)
            .reshape(SHARD, T)
            for cidx in range(NCORES)
        ]
    )
    idx = scores.argmax(1)
    part = np.partition(scores, T - 2, axis=1)
    gap = part[:, -1] - part[:, -2]
    amb = np.nonzero(gap < GAP_THR)[0]
    if len(amb):
        s64 = xf[amb].astype(np.float64) @ sgn.T.astype(np.float64)
        idx[amb] = s64.argmax(1)

    counts = np.bincount(idx, minlength=T)
    a_own, b_own, slots_of = _pack_slots(counts)

    # split each expert's tokens across its slots
    order = np.argsort(idx, kind="stable")
    bounds = np.concatenate([[0], np.cumsum(counts)])
    slot_tokens = {}  # (slot, core) -> token id array
    for t in range(T):
        ids = order[bounds[t] : bounds[t + 1]]
        pos = 0
        for slot, core, cap in slots_of[t]:
            take = min(cap, len(ids) - pos)
            slot_tokens[(slot, core)] = ids[pos : pos + take]
            pos += take
        assert pos == len(ids)

    xfull16 = np.vstack([xf.astype(np.float16), np.zeros((1, D), np.float16)])
    in_maps_b = []
    gl_padded = {}
    for core in range(NCORES):
        gla_in, gla_ids = _wrap_gl(slot_tokens.get(("a", core), []), CAPA)
        glb_in, glb_ids = _wrap_gl(slot_tokens.get(("b", core), []), CAPB)
        gl_padded[core] = (gla_ids, glb_ids)
        wa_in = np.ascontiguousarray(
            W[a_own[core]].astype(np.float16).reshape(DC, 128, D).transpose(1, 0, 2)
        )
        wb_in = np.ascontiguousarray(
            W[b_own[core]].astype(np.float16).reshape(DC, 128, D).transpose(1, 0, 2)
        )
        in_maps_b.append(
            {
                "xfull": xfull16,
                "wa": wa_in,
                "wb": wb_in,
                "ba": b[a_own[core]].reshape(1, D),
                "bb": b[b_own[core]].reshape(1, D),
                "gla": gla_in,
                "glb": glb_in,
            }
        )

    res_b = _run_spmd(nc_b, in_maps_b, "b")

    out_full = np.zeros((NTOK + 1, D), np.float32)
    for core in range(NCORES):
        rows = np.asarray(res_b.results[core]["orows"]).astype(np.float32)
        gla_ids, glb_ids = gl_padded[core]
        out_full[gla_ids] = rows[:CAPA]
        out_full[glb_ids] = rows[CAPA:]
    return out_full[:NTOK].reshape(B, S, D)


# revision 10
# speedup vs baseline: 1.4540x; 1.0836x over previous
"""ContentOnlyRouter MoE kernel for 8x TRN2 NeuronCores.

Strategy (two SPMD launches + host routing glue):
  Launch A (data-parallel scoring): each core scores its 2048-token shard
    against sign(tile_sigs) using fp8-e4m3 x with DoubleRow matmuls (2x PE
    rate), transposes the [8, tok] psum scores into token-partition layout
    and DMAs the raw fp32 scores out. Scores are exact sums of +-fp8(x), so
    the host knows their error bound vs the true fp32 scores.
  Host glue: argmax over device scores; tokens whose top-2 gap < GAP_THR
    are re-scored exactly in float64 (the reference's own fp32-einsum argmax
    matches float64 argmax for every token because the minimum true top-2
    gap is ~1e-3 while fp32 scoring noise is ~6e-5). Then tokens are packed
    into 16 weight slots (8 cores x [A=1152, B=1024] rows) such that each
    slot holds tokens of a single expert - every core runs exactly 17
    blocks of 128 tokens regardless of expert imbalance.
  Launch B (slot-parallel expert matmul): per core, gather its two slots'
    token rows from a replicated fp16 copy of x (dma_gather transpose=True
    into [d, tok] matmul layout), run 16 accumulating fp16 matmuls per
    128-token block against the slot's weights, add bias on DVE, store
    bf16 rows. Host scatters rows back to token order.

Shapes hardcoded for B=4, S=4096, D=1024, T=8 per the problem spec.
"""

import os

os.environ.setdefault("JAX_PLATFORMS", "")

import numpy as np
import ml_dtypes

import concourse.bass as bass
import concourse.bacc as bacc
import concourse.mybir as mybir
import concourse.tile as tile
from concourse.masks import make_identity

B, S, D, T = 4, 4096, 1024, 8
NTOK = B * S            # 16384 tokens
NCORES = 8
SHARD = NTOK // NCORES  # 2048 tokens scored per core
NG = 4                  # score groups of 512 tokens per shard
DC = D // 128           # 8 contraction chunks of 128

CAPA = 1152             # slot A: 9 blocks of 128
CAPB = 1024             # slot B: 8 blocks of 128
CAP = CAPA + CAPB       # 17 blocks per core
TRASH = NTOK            # row index used for padding slots (zero row)
GAP_THR = 6.0           # fp8 score top-2 gap below which host re-scores

F32 = mybir.dt.float32
FP16 = mybir.dt.float16
BF16 = mybir.dt.bfloat16
FP8 = mybir.dt.float8e4
I16 = mybir.dt.int16

NPFP8 = ml_dtypes.float8_e4m3

_perf = []  # (label, exec_time_ns, mean) per launch when tracing


def build_launch_a(iters=1):
    """fp8 DoubleRow scoring for one 2048-token shard; outputs raw scores."""
    nc = bacc.Bacc(None)
    x8 = nc.dram_tensor("x8", [128, DC, SHARD], FP8, kind="ExternalInput")
    sg8 = nc.dram_tensor("sg8", [128, DC, 16], FP8, kind="ExternalInput")
    # sct[p, q, t] = score(token 128*q + p, expert t)
    sct = nc.dram_tensor("sct", [128, SHARD // 128, T], F32, kind="ExternalOutput")

    with tile.TileContext(nc) as tc:
        with (
            tc.tile_pool(name="const", bufs=1) as const,
            tc.tile_pool(name="xa", bufs=2) as xa,
            tc.tile_pool(name="ps", bufs=2, space="PSUM") as ps,
            tc.tile_pool(name="pst", bufs=4, space="PSUM") as pst,
            tc.tile_pool(name="sb", bufs=2) as sb,
        ):
            sg_sb = const.tile([128, DC, 16], FP8)
            nc.sync.dma_start(out=sg_sb, in_=sg8[:, :, :])
            ident = const.tile([128, 128], F32)
            make_identity(nc, ident)
            sc_all = const.tile([128, NG * 4, T], F32)

            import contextlib
            loop = tc.For_i(0, iters, 1) if iters > 1 else contextlib.nullcontext()
            with loop:
                _body_a(nc, tc, xa, ps, pst, sb, sg_sb, ident, sc_all, x8, sct)
    nc.compile()
    return nc


def _body_a(nc, tc, xa, ps, pst, sb, sg_sb, ident, sc_all, x8, sct):
    for g in range(NG):
        xg = xa.tile([128, DC, 512], FP8, tag="xg")
        nc.sync.dma_start(out=xg, in_=x8[:, :, 512 * g : 512 * (g + 1)])
        psum_s = ps.tile([16, 512], F32, tag="psum_s")
        for c in range(DC // 2):
            nc.tensor.matmul(
                out=psum_s,
                lhsT=sg_sb[:, 2 * c : 2 * c + 2, :],
                rhs=xg[:, 2 * c : 2 * c + 2, :],
                perf_mode=mybir.MatmulPerfMode.DoubleRow,
                start=(c == 0),
                stop=(c == DC // 2 - 1),
            )
        s_sb = sb.tile([T, 512], F32, tag="s_sb")
        nc.vector.tensor_copy(out=s_sb, in_=psum_s[0:T, :])
        for j in range(4):
            p_t = pst.tile([128, T], F32, tag="p_t")
            nc.tensor.transpose(
                out=p_t,
                in_=s_sb[:, 128 * j : 128 * (j + 1)],
                identity=ident[0:T, 0:T],
            )
            nc.vector.tensor_copy(out=sc_all[:, 4 * g + j, :], in_=p_t)
        # per-group score store so the tail doesn't serialize after all DMAs
        nc.scalar.dma_start(
            out=sct[:, 4 * g : 4 * (g + 1), :], in_=sc_all[:, 4 * g : 4 * (g + 1), :]
        )


def build_launch_b(iters=1):
    """Gather + two-slot expert matmul (9 + 8 blocks of 128 tokens)."""
    nc = bacc.Bacc(None)
    xfull = nc.dram_tensor("xfull", [NTOK + 1, D], FP16, kind="ExternalInput")
    wa = nc.dram_tensor("wa", [128, DC, D], FP16, kind="ExternalInput")
    wb = nc.dram_tensor("wb", [128, DC, D], FP16, kind="ExternalInput")
    ba = nc.dram_tensor("ba", [1, D], F32, kind="ExternalInput")
    bb = nc.dram_tensor("bb", [1, D], F32, kind="ExternalInput")
    gla = nc.dram_tensor("gla", [128, CAPA // 16], I16, kind="ExternalInput")
    glb = nc.dram_tensor("glb", [128, CAPB // 16], I16, kind="ExternalInput")
    orows = nc.dram_tensor("orows", [CAP, D], BF16, kind="ExternalOutput")

    with tile.TileContext(nc) as tc:
        with (
            tc.tile_pool(name="const", bufs=1) as const,
            tc.tile_pool(name="gxa", bufs=2) as gxa,
            tc.tile_pool(name="gxb", bufs=2) as gxb,
            tc.tile_pool(name="ps", bufs=4, space="PSUM") as ps,
            tc.tile_pool(name="osb", bufs=3) as osb,
        ):
            gla_sb = const.tile([128, CAPA // 16], I16)
            nc.sync.dma_start(out=gla_sb, in_=gla[:, :])
            glb_sb = const.tile([128, CAPB // 16], I16)
            nc.sync.dma_start(out=glb_sb, in_=glb[:, :])

            # bias rows + fp16 ones column for PE partition-broadcast
            ones = const.tile([1, 128], FP16)
            nc.vector.memset(ones, 1.0)
            brow16 = {}
            for slot, src in (("a", ba), ("b", bb)):
                br = const.tile([1, D], F32, tag=f"brow{slot}", name=f"brow{slot}")
                nc.sync.dma_start(out=br, in_=src[:, :])
                br16 = const.tile([1, D], FP16, tag=f"brow16{slot}", name=f"brow16{slot}")
                nc.vector.tensor_copy(out=br16, in_=br)
                brow16[slot] = br16

            # W tiles are loaded on the gpsimd queue interleaved with the
            # gathers (see _body_b) - the modeled DMA belt serves queues with
            # sync > scalar > gpsimd priority, so bulk W on sync would starve
            # the first gather and stall the PE for ~10us.
            w_sb = {}
            w_sb["a"] = const.tile([128, DC, D], FP16, tag="wa_sb", name="wa_sb")
            w_sb["b"] = const.tile([128, DC, D], FP16, tag="wb_sb", name="wb_sb")
            w_dram = {"a": wa, "b": wb}

            # broadcast biases across partitions: ones[1,128].T @ brow16[1,:]
            b_bc = {}
            for slot in ("a", "b"):
                pb0 = ps.tile([128, 512], F32, tag="ps0", name=f"pb0{slot}")
                pb1 = ps.tile([128, 512], F32, tag="ps1", name=f"pb1{slot}")
                nc.tensor.matmul(out=pb0, lhsT=ones,
                                 rhs=brow16[slot][:, 0:512])
                nc.tensor.matmul(out=pb1, lhsT=ones,
                                 rhs=brow16[slot][:, 512:1024])
                bt = const.tile([128, D], F32, tag=f"bbc{slot}", name=f"bbc{slot}")
                nc.vector.tensor_copy(out=bt[:, 0:512], in_=pb0)
                nc.vector.tensor_copy(out=bt[:, 512:1024], in_=pb1)
                b_bc[slot] = bt

            import contextlib
            loop = tc.For_i(0, iters, 1) if iters > 1 else contextlib.nullcontext()
            with loop:
                _body_b(nc, tc, gxa, gxb, ps, osb, w_sb, w_dram, b_bc,
                        gla_sb, glb_sb, xfull, orows)
    nc.compile()
    return nc


def _body_b(nc, tc, gxa, gxb, ps, osb, w_sb, w_dram, b_bc, gla_sb, glb_sb,
            xfull, orows):
    # slot A: 3 gather chunks x 384 tokens (3 blocks); slot B: 2 x 512 (4 blocks)
    plan = [("a", gxa, gla_sb, 384, 3, 0), ("b", gxb, glb_sb, 512, 2, CAPA)]
    gather_no = 0
    for slot, gxp, gl_sb, gchunk, nchunk, row_base in plan:
        for ch in range(nchunk):
            gx = gxp.tile([128, DC, gchunk], FP16, tag=f"gx{slot}", name=f"gx{slot}")
            nc.gpsimd.dma_gather(
                out_ap=gx,
                in_ap=xfull[:, :],
                idxs_ap=gl_sb[:, (gchunk // 16) * ch : (gchunk // 16) * (ch + 1)],
                num_idxs=gchunk,
                num_idxs_reg=gchunk,
                elem_size=D,
                transpose=True,
            )
            # W rides the gpsimd queue right after the gather that precedes
            # its first use: wA chunks after gather 0, wB after gather 1.
            if gather_no == 0:
                for c in range(DC):
                    nc.gpsimd.dma_start(out=w_sb["a"][:, c, :],
                                        in_=w_dram["a"][:, c, :])
            elif gather_no == 1:
                for c in range(DC):
                    nc.gpsimd.dma_start(out=w_sb["b"][:, c, :],
                                        in_=w_dram["b"][:, c, :])
            gather_no += 1
            for blk in range(gchunk // 128):
                tok = slice(128 * blk, 128 * (blk + 1))
                ps0 = ps.tile([128, 512], F32, tag="ps0")
                ps1 = ps.tile([128, 512], F32, tag="ps1")
                for c in range(DC):
                    nc.tensor.matmul(
                        out=ps0,
                        lhsT=gx[:, c, tok],
                        rhs=w_sb[slot][:, c, 0:512],
                        start=(c == 0),
                        stop=(c == DC - 1),
                    )
                    nc.tensor.matmul(
                        out=ps1,
                        lhsT=gx[:, c, tok],
                        rhs=w_sb[slot][:, c, 512:1024],
                        start=(c == 0),
                        stop=(c == DC - 1),
                    )
                o_t = osb.tile([128, D], BF16, tag="o_t")
                nc.vector.tensor_add(out=o_t[:, 0:512], in0=ps0,
                                     in1=b_bc[slot][:, 0:512])
                nc.vector.tensor_add(out=o_t[:, 512:1024], in0=ps1,
                                     in1=b_bc[slot][:, 512:1024])
                row0 = row_base + gchunk * ch + 128 * blk
                nc.scalar.dma_start(out=orows[row0 : row0 + 128, :], in_=o_t)


_nc_a = None
_nc_b = None


def _get_programs():
    global _nc_a, _nc_b
    if _nc_a is None:
        _nc_a = build_launch_a()
        _nc_b = build_launch_b()
    return _nc_a, _nc_b


def _run_spmd(nc, in_maps, label):
    if os.environ.get("BASS_SIM"):
        from concourse.bass_interp import CoreSim

        results = []
        for im in in_maps:
            sim = CoreSim(nc)
            for k, v in im.items():
                sim.tensor(k)[:] = v
            sim.simulate()
            out = {}
            for alloc in nc.m.functions[0].allocations:
                if getattr(alloc, "kind", None) == "ExternalOutput":
                    name = alloc.memorylocations[0].name
                    out[name] = np.array(sim.mem_tensor(name))
            results.append(out)

        class R:
            pass

        r = R()
        r.results = results
        r.exec_time_ns = None
        return r
    from concourse.bass_utils import run_bass_kernel_spmd

    trace = bool(os.environ.get("BASS_TRACE"))
    kw = {}
    if trace:
        tdir = os.path.abspath(f"trace_{label}")
        os.makedirs(tdir, exist_ok=True)
        kw = dict(trace=True, tmpdir=tdir, trace_cores=[0])
    res = run_bass_kernel_spmd(nc, in_maps, core_ids=list(range(NCORES)), **kw)
    if trace:
        _perf.append((label, res.exec_time_ns, res.mean_exec_time_ns))
    return res


def _pack_slots(counts):
    """Assign experts to 16 single-expert slots (8 cores x [A=1152, B=1024]).

    Returns (a_own, b_own): expert id owning each core's A / B slot, plus the
    ordered list of (slot refs) per expert used to split its token list.
    """
    big = [t for t in range(T) if counts[t] > CAPA + CAPB]
    assert all(counts[t] <= 2 * CAPA for t in big), f"expert too large: {counts}"
    # smallest len(big) experts (that are not big) run on two B slots
    order_small = sorted((t for t in range(T) if t not in big),
                         key=lambda t: counts[t])
    bb = order_small[: len(big)]
    assert all(counts[t] <= 2 * CAPB for t in bb), f"B+B overflow: {counts}"
    ab = [t for t in range(T) if t not in big and t not in bb]

    a_own, b_own = [], []
    for t in big:
        a_own += [t, t]
    for t in bb:
        b_own += [t, t]
    for t in ab:
        a_own.append(t)
        b_own.append(t)
    assert len(a_own) == NCORES and len(b_own) == NCORES
    # per-expert ordered slot list: ('a'|'b', core, capacity)
    slots_of = {t: [] for t in range(T)}
    for core, t in enumerate(a_own):
        slots_of[t].append(("a", core, CAPA))
    for core, t in enumerate(b_own):
        slots_of[t].append(("b", core, CAPB))
    for t in range(T):
        assert sum(c for _, _, c in slots_of[t]) >= counts[t], f"pack fail {counts}"
    return a_own, b_own, slots_of


def _wrap_gl(ids, cap):
    glf = np.full(cap, TRASH, np.int64)
    glf[: len(ids)] = ids
    wrapped = np.ascontiguousarray(glf.reshape(cap // 16, 16).T.astype(np.int16))
    return np.tile(wrapped, (8, 1)), glf  # [128, cap//16], padded id list


def kernel(x, tile_sigs, W, b):
    x = np.asarray(x, np.float32)
    tile_sigs = np.asarray(tile_sigs, np.float32)
    W = np.asarray(W, np.float32)
    b = np.asarray(b, np.float32)
    _perf.clear()

    nc_a, nc_b = _get_programs()

    xf = x.reshape(NTOK, D)
    x8 = xf.astype(NPFP8)
    sgn = np.sign(tile_sigs)

    # x8t[p, c, n] = x8[tok0 + n, 128c + p]
    sg8_in = np.zeros((128, DC, 16), NPFP8)
    sg8_in[:, :, :T] = sgn.astype(NPFP8).T.reshape(DC, 128, T).transpose(1, 0, 2)
    in_maps_a = []
    for cidx in range(NCORES):
        sh = slice(cidx * SHARD, (cidx + 1) * SHARD)
        x8t = np.ascontiguousarray(x8[sh].T.reshape(DC, 128, SHARD).transpose(1, 0, 2))
        in_maps_a.append({"x8": x8t, "sg8": sg8_in})

    res_a = _run_spmd(nc_a, in_maps_a, "a")

    # assemble scores: sct[p, q, t] -> scores[2048*core + 128q + p, t]
    scores = np.concatenate(
        [
            np.asarray(res_a.results[cidx]["sct"], np.float32)
            .transpose(1, 0, 2)
            .reshape(SHARD, T)
            for cidx in range(NCORES)
        ]
    )
    idx = scores.argmax(1)
    part = np.partition(scores, T - 2, axis=1)
    gap = part[:, -1] - part[:, -2]
    amb = np.nonzero(gap < GAP_THR)[0]
    if len(amb):
        s64 = xf[amb].astype(np.float64) @ sgn.T.astype(np.float64)
        idx[amb] = s64.argmax(1)

    counts = np.bincount(idx, minlength=T)
    a_own, b_own, slots_of = _pack_slots(counts)

    # split each expert's tokens across its slots
    order = np.argsort(idx, kind="stable")
    bounds = np.concatenate([[0], np.cumsum(counts)])
    slot_tokens = {}  # (slot, core) -> token id array
    for t in range(T):
        ids = order[bounds[t] : bounds[t + 1]]
        pos = 0
        for slot, core, cap in slots_of[t]:
            take = min(cap, len(ids) - pos)
            slot_tokens[(slot, core)] = ids[pos : pos + take]
            pos += take
        assert pos == len(ids)

    xfull16 = np.vstack([xf.astype(np.float16), np.zeros((1, D), np.float16)])
    in_maps_b = []
    gl_padded = {}
    for core in range(NCORES):
        gla_in, gla_ids = _wrap_gl(slot_tokens.get(("a", core), []), CAPA)
        glb_in, glb_ids = _wrap_gl(slot_tokens.get(("b", core), []), CAPB)
        gl_padded[core] = (gla_ids, glb_ids)
        wa_in = np.ascontiguousarray(
            W[a_own[core]].astype(np.float16).reshape(DC, 128, D).transpose(1, 0, 2)
        )
        wb_in = np.ascontiguousarray(
            W[b_own[core]].astype(np.float16).reshape(DC, 128, D).transpose(1, 0, 2)
        )
        in_maps_b.append(
            {
                "xfull": xfull16,
                "wa": wa_in,
                "wb": wb_in,
                "ba": b[a_own[core]].reshape(1, D),
                "bb": b[b_own[core]].reshape(1, D),
                "gla": gla_in,
                "glb": glb_in,
            }
        )

    res_b = _run_spmd(nc_b, in_maps_b, "b")

    out_full = np.zeros((NTOK + 1, D), np.float32)
    for core in range(NCORES):
        rows = np.asarray(res_b.results[core]["orows"]).astype(np.float32)
        gla_ids, glb_ids = gl_padded[core]
        out_full[gla_ids] = rows[:CAPA]
        out_full[glb_ids] = rows[CAPA:]
    return out_full[:NTOK].reshape(B, S, D)
